# revision 19
# baseline (speedup 1.0000x reference)
"""7x7 valid conv2d (cross-correlation) on a 4096x4096 fp32 image, 8 NeuronCores.

Strategy: 2x4 core grid (2 row bands x 4 col bands), halo baked in on the host
so there are no device collectives.  Per core the conv runs on the TensorEngine
as 7 PSUM-accumulated "banded Toeplitz" matmuls per (row-stripe, col-tile):
for each kernel column kx, a [K=128, M=122] stationary matrix T_kx with
T_kx[m+ky, m] = w[ky, kx] contracts 128 input rows into 122 output rows; the
kx shift is a free column offset on the moving operand.

vs the old 1x8 row split this cuts matmuls/core from 280 to 238: each core
runs 17 stripes x 2 col-tiles x 7 kx instead of 5 stripes x 8 col-tiles x 7
(the 8-way row split wasted 16% of PE cycles on a 24-rows-kept 5th stripe).
"""

import numpy as np
import ml_dtypes

import concourse.bacc as bacc
import concourse.bass as bass
import concourse.tile as tile
import concourse.mybir as mybir
from concourse.bass_utils import run_bass_kernel_spmd

H = W = 4096
KH = KW = 7
OH = OW = H - KH + 1  # 4090
NCORES = 8
GR, GC = 2, 4                  # core grid: 2 row bands x 4 col bands
ROWS_PC = 2045                 # output rows per core
COLS_PC = 1023                 # output cols per core (col bands overlap by 2)
ROW_BAND = [0, 2045]
COL_BAND = [0, 1023, 2046, 3067]   # last band overlaps band 2 by 2 cols
MT = 122                       # output rows per stripe (contraction K = 128)
ROW_STARTS = list(range(0, ROWS_PC, MT))  # 17 stripes, last keeps 93 rows
IN_ROWS = ROW_STARTS[-1] + 128            # 2080 (2051 real + pad)
IN_COLS = 1032                            # 1023 + 6 halo + 3 pad
OUT_COLS = 1040                           # 1023 + pad: non-contig DRAM rows keep
                                          # store descriptors sprayed across queues
COL_TILES = [(0, 512), (512, 511)]        # (c0, N) psum col tiles

MODE = "bf16"
TRACE = False
LAST_EXEC_NS = None

_DT = {
    "bf16": (mybir.dt.bfloat16, ml_dtypes.bfloat16),
    "fp32": (mybir.dt.float32, np.float32),
}

_compiled = {}


def _build(mode):
    dt_b, _ = _DT[mode]
    nc = bacc.Bacc(
        "TRN2", target_bir_lowering=False, debug=False, num_devices=NCORES
    )
    x_d = nc.dram_tensor("x", [IN_ROWS, IN_COLS], dt_b, kind="ExternalInput").ap()
    t_d = nc.dram_tensor("tmats", [128, KW * MT], dt_b, kind="ExternalInput").ap()
    # bf16 output staging/store halves store traffic; host upcasts.
    o_d = nc.dram_tensor(
        "out", [ROWS_PC, OUT_COLS], mybir.dt.bfloat16, kind="ExternalOutput"
    ).ap()

    with tile.TileContext(nc) as tc:
        with (
            tc.tile_pool(name="tmat", bufs=1) as tpool,
            tc.tile_pool(name="xsl", bufs=6) as xpool,
            tc.tile_pool(name="ps", bufs=7, space="PSUM") as ppool,
            tc.tile_pool(name="ost", bufs=8) as opool,
        ):
            tm = tpool.tile([128, KW * MT], dt_b)
            tmw = tpool.tile([1, 96], dt_b, name="tmw")
            # tiny warmup operand via SWDGE (parallel to sync's desc-gen)
            nc.gpsimd.dma_start(tmw[:], t_d[:1, :96])
            nc.sync.dma_start(tm[:], t_d[:])
            # ~3.2us of dummy matmuls while the first x chunk is in flight:
            # keeps the PE_HAM activity window busy so the real stream starts
            # at the 2.4 GHz clock instead of ramping from 1.2 GHz
            pw = ppool.tile([1, 512], mybir.dt.float32, tag="warm", bufs=1)
            for _ in range(40):
                nc.tensor.matmul(pw[:1, :96], tmw[:1, :1], tmw[:1, :96])
            for ti, r0 in enumerate(ROW_STARTS):
                M = min(MT, ROWS_PC - r0)  # rows actually kept
                xt = xpool.tile([128, IN_COLS], dt_b, tag="x")
                if ti == 0:
                    # first matmul group only needs cols 0-518: land it early
                    nc.sync.dma_start(xt[:, :524], x_d[r0 : r0 + 128, :524])
                    nc.sync.dma_start(xt[:, 524:], x_d[r0 : r0 + 128, 524:])
                else:
                    nc.sync.dma_start(xt[:, :], x_d[r0 : r0 + 128, :])
                ot = opool.tile([128, COLS_PC], mybir.dt.bfloat16, tag="o")
                for ci, (c0, N) in enumerate(COL_TILES):
                    ps = ppool.tile([MT, 512], mybir.dt.float32, tag="ps")
                    for kx in range(KW):
                        nc.tensor.matmul(
                            ps[:, :N],
                            tm[:, kx * MT : kx * MT + MT],
                            xt[:, c0 + kx : c0 + kx + N],
                            start=(kx == 0),
                            stop=(kx == KW - 1),
                        )
                    # PSUM drain split across DVE and ACT so the two col-tiles'
                    # casts run in parallel (shortens the end-of-stream drain)
                    if ci == 0:
                        nc.vector.tensor_copy(ot[:M, c0 : c0 + N], ps[:M, :N])
                    else:
                        nc.scalar.copy(ot[:M, c0 : c0 + N], ps[:M, :N])
                    # SWDGE (gpsimd) store: sprays descriptors across all 16
                    # SDMA engines. HWDGE funnels this pattern onto 2 queues
                    # (~54 GB/s) - measured, do not switch.  Last stripes ship
                    # per col-tile so the final drain chain is short.
                    if ti >= len(ROW_STARTS) - 3:
                        nc.gpsimd.dma_start(
                            o_d[r0 : r0 + M, c0 : c0 + N], ot[:M, c0 : c0 + N]
                        )
                if ti < len(ROW_STARTS) - 3:
                    nc.gpsimd.dma_start(o_d[r0 : r0 + M, :COLS_PC], ot[:M, :COLS_PC])
    nc.compile()
    return nc


def _toeplitz(weight, np_dt):
    t = np.zeros((128, KW * MT), dtype=np.float32)
    idx = np.arange(MT)
    for kx in range(KW):
        for ky in range(KH):
            t[idx + ky, kx * MT + idx] = weight[ky, kx]
    return np.ascontiguousarray(t.astype(np_dt))


def kernel(x, weight):
    global LAST_EXEC_NS
    mode = MODE
    dt_b, np_dt = _DT[mode]
    if mode not in _compiled:
        _compiled[mode] = _build(mode)
    nc = _compiled[mode]

    xf = np.asarray(x, np.float32)
    wf = np.asarray(weight, np.float32)
    tmats = _toeplitz(wf, np_dt)
    xc = xf.astype(np_dt) if np_dt is not np.float32 else xf

    # padded canvas so every core's slab is [IN_ROWS, IN_COLS]
    xpad = np.zeros((ROW_BAND[-1] + IN_ROWS, COL_BAND[-1] + IN_COLS), dtype=xc.dtype)
    xpad[:H, :W] = xc
    in_maps = []
    for c in range(NCORES):
        r0, c0 = ROW_BAND[c // GC], COL_BAND[c % GC]
        in_maps.append(
            {
                "x": np.ascontiguousarray(xpad[r0 : r0 + IN_ROWS, c0 : c0 + IN_COLS]),
                "tmats": tmats,
            }
        )
    res = run_bass_kernel_spmd(
        nc, in_maps, core_ids=list(range(NCORES)), trace=TRACE
    )
    LAST_EXEC_NS = res.exec_time_ns

    out = np.empty((OH, OW), np.float32)
    for c in range(NCORES):
        r0, c0 = ROW_BAND[c // GC], COL_BAND[c % GC]
        out[r0 : r0 + ROWS_PC, c0 : c0 + COLS_PC] = res.results[c]["out"][
            :, :COLS_PC
        ].astype(np.float32)
    return out


# revision 23
# speedup vs baseline: 1.0684x; 1.0684x over previous
"""7x7 valid conv2d (cross-correlation) on a 4096x4096 fp32 image, 8 NeuronCores.

Strategy: 2x4 core grid (2 row bands x 4 col bands), halo baked in on the host
so there are no device collectives.  Per core the conv runs on the TensorEngine
as 7 PSUM-accumulated "banded Toeplitz" matmuls per (row-stripe, col-tile):
for each kernel column kx, a [K=128, M=122] stationary matrix T_kx with
T_kx[m+ky, m] = w[ky, kx] contracts 128 input rows into 122 output rows; the
kx shift is a free column offset on the moving operand.

vs the old 1x8 row split this cuts matmuls/core from 280 to 238: each core
runs 17 stripes x 2 col-tiles x 7 kx instead of 5 stripes x 8 col-tiles x 7
(the 8-way row split wasted 16% of PE cycles on a 24-rows-kept 5th stripe).
"""

import numpy as np
import ml_dtypes

import concourse.bacc as bacc
import concourse.bass as bass
import concourse.tile as tile
import concourse.mybir as mybir
from concourse.bass_utils import run_bass_kernel_spmd

H = W = 4096
KH = KW = 7
OH = OW = H - KH + 1  # 4090
NCORES = 8
GR, GC = 2, 4                  # core grid: 2 row bands x 4 col bands
ROWS_PC = 2045                 # output rows per core
COLS_PC = 1023                 # output cols per core (col bands overlap by 2)
ROW_BAND = [0, 2045]
COL_BAND = [0, 1023, 2046, 3067]   # last band overlaps band 2 by 2 cols
MT = 122                       # output rows per stripe (contraction K = 128)
ROW_STARTS = list(range(0, ROWS_PC, MT))  # 17 stripes, last keeps 93 rows
IN_ROWS = ROW_STARTS[-1] + 128            # 2080 (2051 real + pad)
IN_COLS = 1032                            # 1023 + 6 halo + 3 pad
OUT_COLS = 1040                           # 1023 + pad: non-contig DRAM rows keep
                                          # store descriptors sprayed across queues
COL_TILES = [(0, 512), (512, 511)]        # (c0, N) psum col tiles

MODE = "bf16"
TRACE = False
LAST_EXEC_NS = None

_DT = {
    "bf16": (mybir.dt.bfloat16, ml_dtypes.bfloat16),
    "fp32": (mybir.dt.float32, np.float32),
}

_compiled = {}


def _build(mode):
    dt_b, _ = _DT[mode]
    nc = bacc.Bacc(
        "TRN2", target_bir_lowering=False, debug=False, num_devices=NCORES
    )
    x_d = nc.dram_tensor("x", [IN_ROWS, IN_COLS], dt_b, kind="ExternalInput").ap()
    t_d = nc.dram_tensor("tmats", [128, KW * MT], dt_b, kind="ExternalInput").ap()
    # bf16 output staging/store halves store traffic; host upcasts.
    o_d = nc.dram_tensor(
        "out", [ROWS_PC, OUT_COLS], mybir.dt.bfloat16, kind="ExternalOutput"
    ).ap()

    with tile.TileContext(nc) as tc:
        with (
            tc.tile_pool(name="tmat", bufs=1) as tpool,
            tc.tile_pool(name="xsl", bufs=6) as xpool,
            tc.tile_pool(name="ps", bufs=7, space="PSUM") as ppool,
            tc.tile_pool(name="ost", bufs=8) as opool,
        ):
            tm = tpool.tile([128, KW * MT], dt_b)
            # full-size dummy matmuls on an uninitialized scratch tile (no DMA
            # dependency, start right at engine-main): keeps the PE_HAM window
            # busy so the real stream starts at 2.4 GHz instead of 1.2
            xw = tpool.tile([128, 640], dt_b, name="xw")
            nc.vector.memset(xw[:], 0)
            pw = ppool.tile([122, 512], mybir.dt.float32, tag="warm", bufs=1)
            for _ in range(8):
                nc.tensor.matmul(pw[:, :512], xw[:, :122], xw[:, 128:640])
            first = True
            for ti, r0 in enumerate(ROW_STARTS):
                M = min(MT, ROWS_PC - r0)  # rows actually kept
                xt = xpool.tile([128, IN_COLS], dt_b, tag="x")
                if ti == 0:
                    # first matmul group only needs cols 0-518: land it early
                    # (and ahead of the tm descriptor-gen on the sync queue)
                    nc.sync.dma_start(xt[:, :524], x_d[r0 : r0 + 128, :524])
                    nc.sync.dma_start(tm[:], t_d[:])
                    nc.sync.dma_start(xt[:, 524:], x_d[r0 : r0 + 128, 524:])
                else:
                    nc.sync.dma_start(xt[:, :], x_d[r0 : r0 + 128, :])
                ot = opool.tile([128, COLS_PC], mybir.dt.bfloat16, tag="o")
                for ci, (c0, N) in enumerate(COL_TILES):
                    ps = ppool.tile([MT, 512], mybir.dt.float32, tag="ps")
                    for kx in range(KW):
                        nc.tensor.matmul(
                            ps[:, :N],
                            tm[:, kx * MT : kx * MT + MT],
                            xt[:, c0 + kx : c0 + kx + N],
                            start=(kx == 0),
                            stop=(kx == KW - 1),
                        )
                    # PSUM drain split across DVE and ACT so the two col-tiles'
                    # casts run in parallel (shortens the end-of-stream drain)
                    if ci == 0:
                        nc.vector.tensor_copy(ot[:M, c0 : c0 + N], ps[:M, :N])
                    else:
                        nc.scalar.copy(ot[:M, c0 : c0 + N], ps[:M, :N])
                    # SWDGE (gpsimd) store: sprays descriptors across all 16
                    # SDMA engines. HWDGE funnels this pattern onto 2 queues
                    # (~54 GB/s) - measured, do not switch.  Last stripes ship
                    # per col-tile so the final drain chain is short.
                    if ti >= len(ROW_STARTS) - 3:
                        nc.gpsimd.dma_start(
                            o_d[r0 : r0 + M, c0 : c0 + N], ot[:M, c0 : c0 + N]
                        )
                if ti < len(ROW_STARTS) - 3:
                    nc.gpsimd.dma_start(o_d[r0 : r0 + M, :COLS_PC], ot[:M, :COLS_PC])
    nc.compile()
    return nc


def _toeplitz(weight, np_dt):
    t = np.zeros((128, KW * MT), dtype=np.float32)
    idx = np.arange(MT)
    for kx in range(KW):
        for ky in range(KH):
            t[idx + ky, kx * MT + idx] = weight[ky, kx]
    return np.ascontiguousarray(t.astype(np_dt))


def kernel(x, weight):
    global LAST_EXEC_NS
    mode = MODE
    dt_b, np_dt = _DT[mode]
    if mode not in _compiled:
        _compiled[mode] = _build(mode)
    nc = _compiled[mode]

    xf = np.asarray(x, np.float32)
    wf = np.asarray(weight, np.float32)
    tmats = _toeplitz(wf, np_dt)
    xc = xf.astype(np_dt) if np_dt is not np.float32 else xf

    # padded canvas so every core's slab is [IN_ROWS, IN_COLS]
    xpad = np.zeros((ROW_BAND[-1] + IN_ROWS, COL_BAND[-1] + IN_COLS), dtype=xc.dtype)
    xpad[:H, :W] = xc
    in_maps = []
    for c in range(NCORES):
        r0, c0 = ROW_BAND[c // GC], COL_BAND[c % GC]
        in_maps.append(
            {
                "x": np.ascontiguousarray(xpad[r0 : r0 + IN_ROWS, c0 : c0 + IN_COLS]),
                "tmats": tmats,
            }
        )
    res = run_bass_kernel_spmd(
        nc, in_maps, core_ids=list(range(NCORES)), trace=TRACE
    )
    LAST_EXEC_NS = res.exec_time_ns

    out = np.empty((OH, OW), np.float32)
    for c in range(NCORES):
        r0, c0 = ROW_BAND[c // GC], COL_BAND[c % GC]
        out[r0 : r0 + ROWS_PC, c0 : c0 + COLS_PC] = res.results[c]["out"][
            :, :COLS_PC
        ].astype(np.float32)
    return out
